# revision 9
# baseline (speedup 1.0000x reference)
"""DGCN diffusion-graph-conv kernel for 8 Trainium2 NeuronCores (v3).

Math (per the reference):
    support S = D^-1/2 (adj+I)^T D^-1/2,  D = diag(rowsum(adj+I))
    x_m = T_m(S) x0  (Chebyshev, K=3),  out = sum_m x_m @ W_m + bias

Folding the Chebyshev coefficients into the weights
    V0 = W0 - W2, V1 = W1 - 3*W3, V2 = 2*W2, V3 = 4*W3
gives out_b = sum_{m=0..3} S^m (X_b @ V_m).

Per-core plan (data-parallel over batch, 4 batches/core):
    1.  Build S^T from adj on-chip; PE-transpose it to get S tiles;
        compute (S^T)^2 and (S^T)^3 in fp32r.
    2.  U0 = X @ V0 + bias in bf16xbf16 matmuls (error-critical path).
    3.  U_m = X @ V_m (m=1..3) in fp8e4m3 with DoubleRow perf mode
        (two 128-deep K-tiles per pass, ~1.7x fp32r throughput).
        Errors here are suppressed ~20x by the later S^m contraction.
    4.  out = U0 + [S|S^2|S^3]-apply over stacked U (fp8 DoubleRow).
All fp8 tensors carry power-of-2 scales (X:16, V:32, U:8, S^m:256);
the combined descale 2^-11 is folded into the final eviction.

Scheduling: x0 loads ride the otherwise-idle tensor-engine DMA queue;
gpsimd's queue is kept clear for the small adj/dscr/dbc control path
that gates all PE work; dummy eye-matmuls warm the PE (HAM clock gate)
before the real stream; eviction work is spread over DVE/ACT/Pool.
"""

import numpy as np
import ml_dtypes

import concourse.bacc as bacc
import concourse.tile as tile
import concourse.mybir as mybir
from concourse.bass_utils import run_bass_kernel_spmd

F32 = mybir.dt.float32
F32R = mybir.dt.float32r
BF16 = mybir.dt.bfloat16
F8 = mybir.dt.float8e4
AX = mybir.AxisListType
ALU = mybir.AluOpType
DR = mybir.MatmulPerfMode.DoubleRow

N_CORES = 8
B, N, D = 32, 512, 768
BL = B // N_CORES          # batches per core = 4
BN = BL * N                # rows per core = 2048
NT = BN // 128             # 16 row blocks
DT = D // 128              # 6 feature tiles
DP = DT // 2               # 3 feature-tile pairs
JT = N // 128              # 4 node tiles
EC = 384                   # output-column chunk (psum-bank safe)

SX, SV, SU, SP = 16.0, 32.0, 8.0, 256.0
DESCALE = 1.0 / (SP * SU)          # 2^-11
U8SCALE = SU / (SX * SV)           # 1/64

WARMUP_A = 40                      # eye x eye matmuls (N=128)
WARMUP_B = 10                      # eye x adj matmuls (N=512)


def _build_program():
    nc = bacc.Bacc("TRN2", target_bir_lowering=False, debug=False,
                   num_devices=N_CORES)
    inpT_d = nc.dram_tensor("inpT", [D, BN], BF16, kind="ExternalInput").ap()
    adj_d = nc.dram_tensor("adj", [N, N], F32, kind="ExternalInput").ap()
    wts_d = nc.dram_tensor("wts", [D * 4, D], F32, kind="ExternalInput").ap()
    bias_d = nc.dram_tensor("bias", [D], F32, kind="ExternalInput").ap()
    eye_d = nc.dram_tensor("eye", [128, 128], F32, kind="ExternalInput").ap()
    out_d = nc.dram_tensor("out", [BN, D], F32, kind="ExternalOutput").ap()
    dscr = nc.dram_tensor("dscr", [N], F32)

    wts_v = wts_d.rearrange("(d m) e -> m d e", m=4)

    with tile.TileContext(nc) as tc:
        with (
            tc.tile_pool(name="const", bufs=1) as constp,
            tc.tile_pool(name="x0", bufs=1) as x0p,
            tc.tile_pool(name="x8", bufs=1) as x8p,
            tc.tile_pool(name="wraw", bufs=8) as wp,
            tc.tile_pool(name="v0", bufs=1) as v0p,
            tc.tile_pool(name="v8", bufs=1) as v8p,
            tc.tile_pool(name="vtmp", bufs=2) as vtp,
            tc.tile_pool(name="sup", bufs=1) as supp,
            tc.tile_pool(name="pt8", bufs=1) as pt8p,
            tc.tile_pool(name="u0", bufs=1) as u0p,
            tc.tile_pool(name="u8", bufs=1) as u8p,
            tc.tile_pool(name="outst", bufs=4) as outp,
            tc.tile_pool(name="psA", bufs=6, space="PSUM") as psA,
            tc.tile_pool(name="psT", bufs=2, space="PSUM") as psT,
        ):
            # ---- x0 loads at the head of the scalar (ACT) DMA queue ----
            x0 = []
            for dt in range(DT):
                x = x0p.tile([128, BN], BF16, name=f"x0_{dt}")
                nc.scalar.dma_start(
                    x[:], inpT_d[dt * 128:(dt + 1) * 128, :])
                x0.append(x)

            # ---- control-path DMAs on a clear gpsimd queue ----
            eye = constp.tile([128, 128], F32R)
            nc.gpsimd.dma_start(eye[:], eye_d[:].bitcast(F32R))
            adjts = []
            for t in range(JT):
                a = supp.tile([128, N], F32, name=f"adj{t}")
                nc.gpsimd.dma_start(a[:], adj_d[t * 128:(t + 1) * 128, :])
                adjts.append(a)

            # ---- weights on sync queue: W0/W2 first (gate V0 -> U0) ----
            wtiles = {}
            for m in (0, 2, 1, 3):
                for dt in range(DT):
                    w = wp.tile([128, D], F32, name=f"w{m}_{dt}", tag="wt")
                    nc.sync.dma_start(
                        w[:], wts_v[m, dt * 128:(dt + 1) * 128, :])
                    wtiles[(m, dt)] = w

            # ---- PE warmup: release the HAM clock gate before real work ----
            for k in range(WARMUP_A):
                wps = psA.tile([128, 128], F32, name=f"wa{k}", tag="ps")
                nc.tensor.matmul(wps[:], eye[:], eye[:], start=True,
                                 stop=True)
            for k in range(WARMUP_B):
                wps = psA.tile([128, 512], F32, name=f"wb{k}", tag="ps")
                nc.tensor.matmul(wps[:], x0[0][:, 0:128], x0[0][:, 0:512],
                                 start=True, stop=True)

            # ---------------- support S^T ----------------
            dcols, dsqs = [], []
            for t in range(JT):
                rs = supp.tile([128, 1], F32, name=f"rs{t}", tag="rs", bufs=2)
                nc.vector.tensor_reduce(rs[:], adjts[t][:], axis=AX.X,
                                        op=ALU.add)
                nc.vector.tensor_scalar_add(rs[:], rs[:], 1.0)
                sq = supp.tile([128, 1], F32, name=f"sq{t}", tag="sq", bufs=2)
                nc.scalar.sqrt(sq[:], rs[:])
                dcol = supp.tile([128, 1], F32, name=f"dcol{t}")
                nc.vector.reciprocal(dcol[:], sq[:])
                dsq = supp.tile([128, 1], F32, name=f"dsq{t}")
                nc.vector.tensor_mul(dsq[:], dcol[:], dcol[:])
                nc.gpsimd.dma_start(dscr.ap()[t * 128:(t + 1) * 128], dcol[:])
                dcols.append(dcol)
                dsqs.append(dsq)
            dbc = constp.tile([128, N], F32)
            nc.gpsimd.dma_start(
                dbc[:], dscr.ap().unsqueeze(0).broadcast_to([128, N]))
            bias_bc = constp.tile([128, D], F32)
            nc.gpsimd.dma_start(
                bias_bc[:], bias_d.unsqueeze(0).broadcast_to([128, D]))
            st_t = []
            for t in range(JT):
                s = supp.tile([128, N], F32R, name=f"st{t}")
                nc.vector.scalar_tensor_tensor(
                    s[:], adjts[t][:], dcols[t][:], dbc[:], ALU.mult, ALU.mult)
                dfix = supp.tile([128, 128], F32, name=f"dfix{t}",
                                 tag="dfix", bufs=2)
                nc.vector.tensor_scalar_mul(dfix[:], eye[:].bitcast(F32),
                                            dsqs[t][:])
                nc.vector.tensor_add(
                    s[:, t * 128:(t + 1) * 128],
                    s[:, t * 128:(t + 1) * 128], dfix[:])
                st_t.append(s)

            # ---------------- V0 (bf16) ----------------
            v0 = []
            for dt in range(DT):
                v = v0p.tile([128, D], BF16, name=f"v0_{dt}")
                nc.vector.tensor_sub(v[:], wtiles[(0, dt)][:],
                                     wtiles[(2, dt)][:])
                v0.append(v)

            # ---------------- X8 quantize (x0 * 16 -> fp8) ----------------
            x8 = []
            engs = [nc.vector, nc.scalar]
            for dp in range(DP):
                t8 = x8p.tile([128, 2, BN], F8, name=f"x8_{dp}")
                for i in range(2):
                    if (2 * dp + i) % 2 == 1:
                        nc.scalar.mul(t8[:, i, :], x0[2 * dp + i][:], SX)
                    else:
                        nc.vector.tensor_scalar_mul(
                            t8[:, i, :], x0[2 * dp + i][:], SX)
                x8.append(t8)

            # ---- V1 tmp on gpsimd; V8 quantize on scalar ----
            v8 = {}
            for m in (1, 2, 3):
                for dp in range(DP):
                    v8[(m, dp)] = v8p.tile([128, 2, D], F8,
                                           name=f"v8_{m}_{dp}")
            for dp in range(DP):
                for i in range(2):
                    dt = 2 * dp + i
                    tmp = vtp.tile([128, D], F32, name=f"vt_{dt}", tag="vt")
                    nc.vector.scalar_tensor_tensor(
                        tmp[:], wtiles[(3, dt)][:], -3.0, wtiles[(1, dt)][:],
                        ALU.mult, ALU.add)
                    nc.scalar.mul(v8[(1, dp)][:, i, :], tmp[:], SV)
                    nc.scalar.mul(v8[(2, dp)][:, i, :], wtiles[(2, dt)][:],
                                  2.0 * SV)
                    nc.scalar.mul(v8[(3, dp)][:, i, :], wtiles[(3, dt)][:],
                                  4.0 * SV)

            # ---------------- PE: transpose S^T -> S ----------------
            s_t = [supp.tile([128, N], F32R, name=f"s{t}") for t in range(JT)]
            for src in range(JT):
                for dst in range(JT):
                    pt = psT.tile([128, 128], F32R, name=f"pt{src}_{dst}",
                                  tag="pt")
                    nc.tensor.transpose(
                        pt[:], st_t[src][:, dst * 128:(dst + 1) * 128], eye[:])
                    nc.scalar.copy(s_t[dst][:, src * 128:(src + 1) * 128],
                                   pt[:])

            # ---------------- PE: powers (S^T)^2, (S^T)^3 ----------------
            pt8 = {}
            for m in (1, 2, 3):
                for u in range(2):
                    pt8[(m, u)] = pt8p.tile([128, 2, N], F8,
                                            name=f"pt8_{m}_{u}")
            for u in range(2):
                for i in range(2):
                    nc.vector.tensor_scalar_mul(
                        pt8[(1, u)][:, i, :], st_t[2 * u + i][:], SP)
            st2 = [supp.tile([128, N], F32R, name=f"st2_{t}")
                   for t in range(JT)]
            for ab in range(JT):
                ps = psA.tile([128, 512], F32, name=f"p2_{ab}", tag="ps")
                for cb in range(JT):
                    nc.tensor.matmul(
                        ps[:], s_t[cb][:, ab * 128:(ab + 1) * 128],
                        st_t[cb][:], start=(cb == 0), stop=(cb == JT - 1))
                nc.scalar.copy(st2[ab][:], ps[:])
                nc.vector.tensor_scalar_mul(
                    pt8[(2, ab // 2)][:, ab % 2, :], ps[:], SP)
            for ab in range(JT):
                ps = psA.tile([128, 512], F32, name=f"p3_{ab}", tag="ps")
                for cb in range(JT):
                    nc.tensor.matmul(
                        ps[:], s_t[cb][:, ab * 128:(ab + 1) * 128],
                        st2[cb][:], start=(cb == 0), stop=(cb == JT - 1))
                nc.vector.tensor_scalar_mul(
                    pt8[(3, ab // 2)][:, ab % 2, :], ps[:], SP)

            # ---------------- U0 = X @ V0 + bias (bf16) ----------------
            u0 = []
            for rb in range(NT):
                ut = u0p.tile([128, D], BF16, name=f"u0_{rb}")
                u0.append(ut)
                pss = [psA.tile([128, 512], F32, name=f"pu0_{rb}_{e}",
                                tag="ps") for e in range(2)]
                for dt in range(DT):
                    lhs = x0[dt][:, rb * 128:(rb + 1) * 128]
                    for e in range(2):
                        nc.tensor.matmul(
                            pss[e][:, 0:EC], lhs,
                            v0[dt][:, e * EC:(e + 1) * EC],
                            start=(dt == 0), stop=(dt == DT - 1))
                for e in range(2):
                    nc.vector.tensor_add(
                        ut[:, e * EC:(e + 1) * EC], pss[e][:, 0:EC],
                        bias_bc[:, e * EC:(e + 1) * EC])

            # ---------------- U_m = X @ V_m (fp8 DoubleRow) ----------------
            u8 = {}
            for m in (1, 2, 3):
                for b in range(BL):
                    for u in range(2):
                        u8[(m, b, u)] = u8p.tile(
                            [128, 2, D], F8, name=f"u8_{m}_{b}_{u}")
            for rb in range(NT):
                b, jt = rb // JT, rb % JT
                u, i = jt // 2, jt % 2
                for m in (1, 2, 3):
                    pss = [psA.tile([128, 512], F32, name=f"pu{m}_{rb}_{e}",
                                    tag="ps") for e in range(2)]
                    for dp in range(DP):
                        lhs = x8[dp][:, :, rb * 128:(rb + 1) * 128]
                        for e in range(2):
                            nc.tensor.matmul(
                                pss[e][:, 0:EC], lhs,
                                v8[(m, dp)][:, :, e * EC:(e + 1) * EC],
                                start=(dp == 0), stop=(dp == DP - 1),
                                perf_mode=DR)
                    dst = u8[(m, b, u)]
                    ev = engs[(rb * 3 + m) % 2]
                    for e in range(2):
                        if ev is nc.scalar:
                            ev.mul(dst[:, i, e * EC:(e + 1) * EC],
                                   pss[e][:, 0:EC], U8SCALE)
                        else:
                            ev.tensor_scalar_mul(
                                dst[:, i, e * EC:(e + 1) * EC],
                                pss[e][:, 0:EC], U8SCALE)

            # ---------------- apply + final eviction ----------------
            MP = [(1, 0), (1, 1), (2, 0), (2, 1), (3, 0), (3, 1)]
            for b in range(BL):
                for nb in range(JT):
                    rb = b * JT + nb
                    pss = [psA.tile([128, 512], F32, name=f"pa_{rb}_{e}",
                                    tag="ps") for e in range(2)]
                    for k, (m, u) in enumerate(MP):
                        lhs = pt8[(m, u)][:, :, nb * 128:(nb + 1) * 128]
                        for e in range(2):
                            nc.tensor.matmul(
                                pss[e][:, 0:EC], lhs,
                                u8[(m, b, u)][:, :, e * EC:(e + 1) * EC],
                                start=(k == 0), stop=(k == len(MP) - 1),
                                perf_mode=DR)
                    so = outp.tile([128, D], F32, name=f"so_{rb}", tag="so")
                    ev = nc.vector
                    for e in range(2):
                        ev.scalar_tensor_tensor(
                            so[:, e * EC:(e + 1) * EC], pss[e][:, 0:EC],
                            DESCALE, u0[rb][:, e * EC:(e + 1) * EC],
                            ALU.mult, ALU.add)
                    nc.sync.dma_start(
                        out_d[rb * 128:(rb + 1) * 128, :], so[:])
    nc.compile()
    return nc


_CACHE = {}


def _get_program():
    if "nc" not in _CACHE:
        _CACHE["nc"] = _build_program()
    return _CACHE["nc"]


def make_in_maps(inputs, adj, weights, biases):
    inputs = np.ascontiguousarray(inputs, dtype=np.float32)
    adj = np.ascontiguousarray(adj, dtype=np.float32)
    weights = np.ascontiguousarray(weights, dtype=np.float32)
    biases = np.ascontiguousarray(biases, dtype=np.float32)
    assert inputs.shape == (B, N, D)
    assert adj.shape == (N, N)
    assert weights.shape == (D * 4, D)
    assert biases.shape == (D,)
    eye = np.eye(128, dtype=np.float32)
    in_maps = []
    for c in range(N_CORES):
        x0T = np.ascontiguousarray(
            inputs[c * BL:(c + 1) * BL].reshape(BN, D).T).astype(
                ml_dtypes.bfloat16)
        in_maps.append({
            "inpT": x0T,
            "adj": adj,
            "wts": weights,
            "bias": biases,
            "eye": eye,
        })
    return in_maps


def kernel(inputs, adj, weights, biases):
    nc = _get_program()
    in_maps = make_in_maps(inputs, adj, weights, biases)
    res = run_bass_kernel_spmd(nc, in_maps, list(range(N_CORES)))
    out = np.concatenate(
        [res.results[c]["out"].reshape(BL, N, D) for c in range(N_CORES)],
        axis=0)
    return out


# revision 19
# speedup vs baseline: 1.2271x; 1.2271x over previous
"""DGCN diffusion-graph-conv kernel for 8 Trainium2 NeuronCores (v3).

Math (per the reference):
    support S = D^-1/2 (adj+I)^T D^-1/2,  D = diag(rowsum(adj+I))
    x_m = T_m(S) x0  (Chebyshev, K=3),  out = sum_m x_m @ W_m + bias

Folding the Chebyshev coefficients into the weights
    V0 = W0 - W2, V1 = W1 - 3*W3, V2 = 2*W2, V3 = 4*W3
gives out_b = sum_{m=0..3} S^m (X_b @ V_m).

Per-core plan (data-parallel over batch, 4 batches/core):
    1.  Build S^T from adj on-chip; PE-transpose it to get S tiles;
        compute (S^T)^2 and (S^T)^3 in fp32r.
    2.  U0 = X @ V0 + bias in bf16xbf16 matmuls (error-critical path).
    3.  U_m = X @ V_m (m=1..3) in fp8e4m3 with DoubleRow perf mode
        (two 128-deep K-tiles per pass, ~1.7x fp32r throughput).
        Errors here are suppressed ~20x by the later S^m contraction.
    4.  out = U0 + [S|S^2|S^3]-apply over stacked U (fp8 DoubleRow).
All fp8 tensors carry power-of-2 scales (X:16, V:32, U:8, S^m:256);
the combined descale 2^-11 is folded into the final eviction.

Scheduling: x0 loads ride the otherwise-idle tensor-engine DMA queue;
gpsimd's queue is kept clear for the small adj/dscr/dbc control path
that gates all PE work; dummy eye-matmuls warm the PE (HAM clock gate)
before the real stream; eviction work is spread over DVE/ACT/Pool.
"""

import numpy as np
import ml_dtypes

import concourse.bacc as bacc
import concourse.tile as tile
import concourse.mybir as mybir
from concourse.bass_utils import run_bass_kernel_spmd

F32 = mybir.dt.float32
F32R = mybir.dt.float32r
BF16 = mybir.dt.bfloat16
F8 = mybir.dt.float8e4
AX = mybir.AxisListType
ALU = mybir.AluOpType
DR = mybir.MatmulPerfMode.DoubleRow

N_CORES = 8
B, N, D = 32, 512, 768
BL = B // N_CORES          # batches per core = 4
BN = BL * N                # rows per core = 2048
NT = BN // 128             # 16 row blocks
DT = D // 128              # 6 feature tiles
DP = DT // 2               # 3 feature-tile pairs
JT = N // 128              # 4 node tiles
EC = 384                   # output-column chunk (psum-bank safe)

SX, SV, SU, SP = 16.0, 32.0, 8.0, 256.0
DESCALE = 1.0 / (SP * SU)          # 2^-11
U8SCALE = SU / (SX * SV)           # 1/64

WARMUP_A = 40                      # eye x eye matmuls (N=128)
WARMUP_B = 10                      # eye x adj matmuls (N=512)


def _build_program():
    nc = bacc.Bacc("TRN2", target_bir_lowering=False, debug=False,
                   num_devices=N_CORES)
    inpT_d = nc.dram_tensor("inpT", [D, BN], BF16, kind="ExternalInput").ap()
    adj_d = nc.dram_tensor("adj", [N, N], F32, kind="ExternalInput").ap()
    wts_d = nc.dram_tensor("wts", [D * 4, D], F32, kind="ExternalInput").ap()
    bias_d = nc.dram_tensor("bias", [D], F32, kind="ExternalInput").ap()
    eye_d = nc.dram_tensor("eye", [128, 128], F32, kind="ExternalInput").ap()
    out_d = nc.dram_tensor("out", [BN, D], F32, kind="ExternalOutput").ap()
    dscr = nc.dram_tensor("dscr", [N], F32)

    wts_v = wts_d.rearrange("(d m) e -> m d e", m=4)

    with tile.TileContext(nc) as tc:
        with (
            tc.tile_pool(name="const", bufs=1) as constp,
            tc.tile_pool(name="x0", bufs=1) as x0p,
            tc.tile_pool(name="x8", bufs=1) as x8p,
            tc.tile_pool(name="wraw", bufs=8) as wp,
            tc.tile_pool(name="v0", bufs=1) as v0p,
            tc.tile_pool(name="v8", bufs=1) as v8p,
            tc.tile_pool(name="vtmp", bufs=2) as vtp,
            tc.tile_pool(name="sup", bufs=1) as supp,
            tc.tile_pool(name="pt8", bufs=1) as pt8p,
            tc.tile_pool(name="u0", bufs=1) as u0p,
            tc.tile_pool(name="u8", bufs=1) as u8p,
            tc.tile_pool(name="outst", bufs=3) as outp,
            tc.tile_pool(name="psA", bufs=6, space="PSUM") as psA,
            tc.tile_pool(name="psT", bufs=2, space="PSUM") as psT,
        ):
            # ---- x0 loads at the head of the scalar (ACT) DMA queue ----
            x0 = []
            for dt in range(DT):
                x = x0p.tile([128, BN], BF16, name=f"x0_{dt}")
                nc.scalar.dma_start(
                    x[:], inpT_d[dt * 128:(dt + 1) * 128, :])
                x0.append(x)

            # ---- control-path DMAs on a clear gpsimd queue ----
            eye = constp.tile([128, 128], F32R)
            nc.gpsimd.dma_start(eye[:], eye_d[:].bitcast(F32R))
            adjts = []
            for t in range(JT):
                a = supp.tile([128, N], F32, name=f"adj{t}", tag="sb512",
                              bufs=4)
                nc.gpsimd.dma_start(a[:], adj_d[t * 128:(t + 1) * 128, :])
                adjts.append(a)

            # ---- weights on sync queue: W0/W2 first (gate V0 -> U0) ----
            wtiles = {}
            for m in (0, 2, 1, 3):
                for dt in range(DT):
                    w = wp.tile([128, D], F32, name=f"w{m}_{dt}", tag="wt")
                    nc.sync.dma_start(
                        w[:], wts_v[m, dt * 128:(dt + 1) * 128, :])
                    wtiles[(m, dt)] = w

            # ---- PE warmup: release the HAM clock gate before real work ----
            for k in range(WARMUP_A):
                wps = psA.tile([128, 128], F32, name=f"wa{k}", tag="ps")
                nc.tensor.matmul(wps[:], eye[:], eye[:], start=True,
                                 stop=True)
            for k in range(WARMUP_B):
                wps = psA.tile([128, 512], F32, name=f"wb{k}", tag="ps")
                nc.tensor.matmul(wps[:], x0[0][:, 0:128], x0[0][:, 0:512],
                                 start=True, stop=True)

            # ---------------- support S^T ----------------
            dcols, dsqs = [], []
            for t in range(JT):
                rs = supp.tile([128, 1], F32, name=f"rs{t}", tag="rs", bufs=2)
                nc.vector.tensor_reduce(rs[:], adjts[t][:], axis=AX.X,
                                        op=ALU.add)
                nc.vector.tensor_scalar_add(rs[:], rs[:], 1.0)
                sq = supp.tile([128, 1], F32, name=f"sq{t}", tag="sq", bufs=2)
                nc.scalar.sqrt(sq[:], rs[:])
                dcol = supp.tile([128, 1], F32, name=f"dcol{t}")
                nc.vector.reciprocal(dcol[:], sq[:])
                dsq = supp.tile([128, 1], F32, name=f"dsq{t}")
                nc.vector.tensor_mul(dsq[:], dcol[:], dcol[:])
                nc.gpsimd.dma_start(dscr.ap()[t * 128:(t + 1) * 128], dcol[:])
                dcols.append(dcol)
                dsqs.append(dsq)
            dbc = constp.tile([128, N], F32)
            nc.gpsimd.dma_start(
                dbc[:], dscr.ap().unsqueeze(0).broadcast_to([128, N]))
            bias_bc = constp.tile([128, D], F32)
            nc.gpsimd.dma_start(
                bias_bc[:], bias_d.unsqueeze(0).broadcast_to([128, D]))
            st_t = []
            for t in range(JT):
                s = supp.tile([128, N], F32R, name=f"st{t}")
                nc.vector.scalar_tensor_tensor(
                    s[:], adjts[t][:], dcols[t][:], dbc[:], ALU.mult, ALU.mult)
                dfix = supp.tile([128, 128], F32, name=f"dfix{t}",
                                 tag="dfix", bufs=2)
                nc.vector.tensor_scalar_mul(dfix[:], eye[:].bitcast(F32),
                                            dsqs[t][:])
                nc.vector.tensor_add(
                    s[:, t * 128:(t + 1) * 128],
                    s[:, t * 128:(t + 1) * 128], dfix[:])
                st_t.append(s)

            # ---------------- PE: transpose S^T -> S ----------------
            s_t = [supp.tile([128, N], F32R, name=f"s{t}") for t in range(JT)]
            for src in range(JT):
                for dst in range(JT):
                    pt = psT.tile([128, 128], F32R, name=f"pt{src}_{dst}",
                                  tag="pt")
                    nc.tensor.transpose(
                        pt[:], st_t[src][:, dst * 128:(dst + 1) * 128], eye[:])
                    nc.scalar.copy(s_t[dst][:, src * 128:(src + 1) * 128],
                                   pt[:])

            # ---------------- PE: powers (S^T)^2, (S^T)^3 ----------------
            pt8 = {}
            for m in (1, 2, 3):
                for u in range(2):
                    pt8[(m, u)] = pt8p.tile([128, 2, N], F8,
                                            name=f"pt8_{m}_{u}")
            st2 = [supp.tile([128, N], F32R, name=f"st2_{t}", tag="sb512",
                             bufs=4)
                   for t in range(JT)]
            for ab in range(JT):
                ps = psA.tile([128, 512], F32, name=f"p2_{ab}", tag="ps")
                for cb in range(JT):
                    nc.tensor.matmul(
                        ps[:], s_t[cb][:, ab * 128:(ab + 1) * 128],
                        st_t[cb][:], start=(cb == 0), stop=(cb == JT - 1))
                nc.scalar.copy(st2[ab][:], ps[:])
                nc.scalar.mul(pt8[(2, ab // 2)][:, ab % 2, :], ps[:], SP)
            for ab in range(JT):
                ps = psA.tile([128, 512], F32, name=f"p3_{ab}", tag="ps")
                for cb in range(JT):
                    nc.tensor.matmul(
                        ps[:], s_t[cb][:, ab * 128:(ab + 1) * 128],
                        st2[cb][:], start=(cb == 0), stop=(cb == JT - 1))
                nc.scalar.mul(pt8[(3, ab // 2)][:, ab % 2, :], ps[:], SP)

            # ---- V0 (bf16) + V8 m=2 quantize on vector (frees W fast) ----
            v8 = {}
            for m in (1, 2, 3):
                for dp in range(DP):
                    v8[(m, dp)] = v8p.tile([128, 2, D], F8,
                                           name=f"v8_{m}_{dp}")
            v0 = []
            for dt in range(DT):
                v = v0p.tile([128, D], BF16, name=f"v0_{dt}")
                nc.vector.tensor_sub(v[:], wtiles[(0, dt)][:],
                                     wtiles[(2, dt)][:])
                nc.vector.tensor_scalar_mul(
                    v8[(2, dt // 2)][:, dt % 2, :], wtiles[(2, dt)][:],
                    2.0 * SV)
                v0.append(v)

            # ---- PT8 m=1 + V1 tmps (vector) ----
            for u in range(2):
                for i in range(2):
                    nc.vector.tensor_scalar_mul(
                        pt8[(1, u)][:, i, :], st_t[2 * u + i][:], SP)
            vtmps = []
            for dt in range(DT):
                tmp = vtp.tile([128, D], BF16, name=f"vt_{dt}", tag="vt",
                               bufs=6)
                nc.vector.scalar_tensor_tensor(
                    tmp[:], wtiles[(3, dt)][:], -3.0, wtiles[(1, dt)][:],
                    ALU.mult, ALU.add)
                vtmps.append(tmp)

            # ---- X8 quantize on scalar; V8 m=1,3 quantize on scalar ----
            x8 = []
            for dp in range(DP):
                t8 = x8p.tile([128, 2, BN], F8, name=f"x8_{dp}")
                for i in range(2):
                    nc.scalar.mul(t8[:, i, :], x0[2 * dp + i][:], SX)
                x8.append(t8)
            for dp in range(DP):
                for i in range(2):
                    dt = 2 * dp + i
                    nc.scalar.mul(v8[(1, dp)][:, i, :], vtmps[dt][:], SV)
                    nc.scalar.mul(v8[(3, dp)][:, i, :], wtiles[(3, dt)][:],
                                  4.0 * SV)

            # ---------------- U0 = X @ V0 + bias (bf16) ----------------
            u0 = []
            for rb in range(NT):
                ut = u0p.tile([128, D], BF16, name=f"u0_{rb}")
                u0.append(ut)
                pss = [psA.tile([128, 512], F32, name=f"pu0_{rb}_{e}",
                                tag="ps") for e in range(2)]
                for dt in range(DT):
                    lhs = x0[dt][:, rb * 128:(rb + 1) * 128]
                    for e in range(2):
                        nc.tensor.matmul(
                            pss[e][:, 0:EC], lhs,
                            v0[dt][:, e * EC:(e + 1) * EC],
                            start=(dt == 0), stop=(dt == DT - 1))
                for e in range(2):
                    nc.vector.tensor_add(
                        ut[:, e * EC:(e + 1) * EC], pss[e][:, 0:EC],
                        bias_bc[:, e * EC:(e + 1) * EC])

            # ---------------- U_m = X @ V_m (fp8 DoubleRow) ----------------
            u8 = {}
            for m in (1, 2, 3):
                for b in range(BL):
                    for u in range(2):
                        u8[(m, b, u)] = u8p.tile(
                            [128, 2, D], F8, name=f"u8_{m}_{b}_{u}")
            for rb in range(NT):
                b, jt = rb // JT, rb % JT
                u, i = jt // 2, jt % 2
                for m in (1, 2, 3):
                    pss = [psA.tile([128, 512], F32, name=f"pu{m}_{rb}_{e}",
                                    tag="ps") for e in range(2)]
                    for dp in range(DP):
                        lhs = x8[dp][:, :, rb * 128:(rb + 1) * 128]
                        for e in range(2):
                            nc.tensor.matmul(
                                pss[e][:, 0:EC], lhs,
                                v8[(m, dp)][:, :, e * EC:(e + 1) * EC],
                                start=(dp == 0), stop=(dp == DP - 1),
                                perf_mode=DR)
                    dst = u8[(m, b, u)]
                    ev = (nc.vector, nc.scalar)[(rb * 3 + m) % 2]
                    for e in range(2):
                        if ev is nc.scalar:
                            ev.mul(dst[:, i, e * EC:(e + 1) * EC],
                                   pss[e][:, 0:EC], U8SCALE)
                        else:
                            ev.tensor_scalar_mul(
                                dst[:, i, e * EC:(e + 1) * EC],
                                pss[e][:, 0:EC], U8SCALE)

            # ---------------- apply + final eviction ----------------
            MP = [(1, 0), (1, 1), (2, 0), (2, 1), (3, 0), (3, 1)]
            for b in range(BL):
                for nb in range(JT):
                    rb = b * JT + nb
                    pss = [psA.tile([128, 512], F32, name=f"pa_{rb}_{e}",
                                    tag="ps") for e in range(2)]
                    for k, (m, u) in enumerate(MP):
                        lhs = pt8[(m, u)][:, :, nb * 128:(nb + 1) * 128]
                        for e in range(2):
                            nc.tensor.matmul(
                                pss[e][:, 0:EC], lhs,
                                u8[(m, b, u)][:, :, e * EC:(e + 1) * EC],
                                start=(k == 0), stop=(k == len(MP) - 1),
                                perf_mode=DR)
                    so = outp.tile([128, D], F32, name=f"so_{rb}", tag="so")
                    ev = nc.vector
                    for e in range(2):
                        ev.scalar_tensor_tensor(
                            so[:, e * EC:(e + 1) * EC], pss[e][:, 0:EC],
                            DESCALE, u0[rb][:, e * EC:(e + 1) * EC],
                            ALU.mult, ALU.add)
                    nc.sync.dma_start(
                        out_d[rb * 128:(rb + 1) * 128, :], so[:])
    nc.compile()
    return nc


_CACHE = {}


def _get_program():
    if "nc" not in _CACHE:
        _CACHE["nc"] = _build_program()
    return _CACHE["nc"]


def make_in_maps(inputs, adj, weights, biases):
    inputs = np.ascontiguousarray(inputs, dtype=np.float32)
    adj = np.ascontiguousarray(adj, dtype=np.float32)
    weights = np.ascontiguousarray(weights, dtype=np.float32)
    biases = np.ascontiguousarray(biases, dtype=np.float32)
    assert inputs.shape == (B, N, D)
    assert adj.shape == (N, N)
    assert weights.shape == (D * 4, D)
    assert biases.shape == (D,)
    eye = np.eye(128, dtype=np.float32)
    in_maps = []
    for c in range(N_CORES):
        x0T = np.ascontiguousarray(
            inputs[c * BL:(c + 1) * BL].reshape(BN, D).T).astype(
                ml_dtypes.bfloat16)
        in_maps.append({
            "inpT": x0T,
            "adj": adj,
            "wts": weights,
            "bias": biases,
            "eye": eye,
        })
    return in_maps


def kernel(inputs, adj, weights, biases):
    nc = _get_program()
    in_maps = make_in_maps(inputs, adj, weights, biases)
    res = run_bass_kernel_spmd(nc, in_maps, list(range(N_CORES)))
    out = np.concatenate(
        [res.results[c]["out"].reshape(BL, N, D) for c in range(N_CORES)],
        axis=0)
    return out


# revision 21
# speedup vs baseline: 1.2449x; 1.0145x over previous
"""DGCN diffusion-graph-conv kernel for 8 Trainium2 NeuronCores (v5).

Math (per the reference):
    support S = D^-1/2 (adj+I)^T D^-1/2,  D = diag(rowsum(adj+I))
    x_m = T_m(S) x0  (Chebyshev, K=3),  out = sum_m x_m @ W_m + bias

Folding the Chebyshev coefficients into the weights
    V0 = W0 - W2, V1 = W1 - 3*W3, V2 = 2*W2, V3 = 4*W3
gives out_b = sum_{m=0..3} S^m (X_b @ V_m).

Per-core plan (data-parallel over batch, 4 batches/core):
    1.  Build S^T from adj on-chip; PE-transpose it to get S tiles;
        compute (S^T)^2 and (S^T)^3 in fp32r.
    2.  U0 = X @ V0 + bias in bf16xbf16 matmuls (error-critical path).
    3.  U_m = X @ V_m (m=1..3) in fp8e4m3 with DoubleRow perf mode
        (two 128-deep K-tiles per pass, ~1.7x fp32r throughput).
        Errors here are suppressed ~20x by the later S^m contraction.
    4.  out = U0 + [S|S^2|S^3]-apply over stacked U (fp8 DoubleRow).
All fp8 tensors carry power-of-2 scales (X:16, V:32, U:8, S^m:256);
the combined descale 2^-11 is folded into the final eviction.

Scheduling notes:
  - The d_j row broadcast is built on-chip (PE transpose of the d
    column + one-hot matmul) instead of a DRAM round trip, whose DMA
    latency under load (~17us each way) used to gate all PE work.
  - adj ships as bf16 (halves the early DMA bytes; S error from it is
    suppressed by the small magnitude of the S^m terms).
  - Weight DMAs are ordered pairwise (W0,W2 interleaved, then W1,W3)
    so tile-pool rotation never blocks an urgent load behind an
    unconsumed one.
  - Dummy eye-matmuls warm the PE (HAM clock gate) before real work.
"""

import numpy as np
import ml_dtypes

import concourse.bacc as bacc
import concourse.tile as tile
import concourse.mybir as mybir
from concourse.bass_utils import run_bass_kernel_spmd

F32 = mybir.dt.float32
F32R = mybir.dt.float32r
BF16 = mybir.dt.bfloat16
F8 = mybir.dt.float8e4
AX = mybir.AxisListType
ALU = mybir.AluOpType
DR = mybir.MatmulPerfMode.DoubleRow

N_CORES = 8
B, N, D = 32, 512, 768
BL = B // N_CORES          # batches per core = 4
BN = BL * N                # rows per core = 2048
NT = BN // 128             # 16 row blocks
DT = D // 128              # 6 feature tiles
DP = DT // 2               # 3 feature-tile pairs
JT = N // 128              # 4 node tiles
EC = 384                   # output-column chunk (psum-bank safe)

SX, SV, SU, SP = 16.0, 32.0, 8.0, 256.0
DESCALE = 1.0 / (SP * SU)          # 2^-11
U8SCALE = SU / (SX * SV)           # 1/64

WARMUP_A = 40                      # eye x eye matmuls (N=128)
WARMUP_B = 10                      # bf16 junk matmuls on x0 (N=512)


def _build_program():
    nc = bacc.Bacc("TRN2", target_bir_lowering=False, debug=False,
                   num_devices=N_CORES)
    inpT_d = nc.dram_tensor("inpT", [D, BN], BF16, kind="ExternalInput").ap()
    adj_d = nc.dram_tensor("adj8", [N, N], BF16, kind="ExternalInput").ap()
    wts_d = nc.dram_tensor("wts", [D * 4, D], F32, kind="ExternalInput").ap()
    bias_d = nc.dram_tensor("bias", [D], F32, kind="ExternalInput").ap()
    eye_d = nc.dram_tensor("eye", [128, 128], F32, kind="ExternalInput").ap()
    oh_d = nc.dram_tensor("oh", [JT, N], F32, kind="ExternalInput").ap()
    out_d = nc.dram_tensor("out", [BN, D], F32, kind="ExternalOutput").ap()

    wts_v = wts_d.rearrange("(d m) e -> m d e", m=4)

    with tile.TileContext(nc) as tc:
        with (
            tc.tile_pool(name="const", bufs=1) as constp,
            tc.tile_pool(name="x0", bufs=1) as x0p,
            tc.tile_pool(name="x8", bufs=1) as x8p,
            tc.tile_pool(name="wraw", bufs=8) as wp,
            tc.tile_pool(name="v0", bufs=1) as v0p,
            tc.tile_pool(name="v8", bufs=1) as v8p,
            tc.tile_pool(name="vtmp", bufs=2) as vtp,
            tc.tile_pool(name="sup", bufs=1) as supp,
            tc.tile_pool(name="pt8", bufs=1) as pt8p,
            tc.tile_pool(name="u0", bufs=1) as u0p,
            tc.tile_pool(name="u8", bufs=1) as u8p,
            tc.tile_pool(name="outst", bufs=3) as outp,
            tc.tile_pool(name="psA", bufs=6, space="PSUM") as psA,
            tc.tile_pool(name="psT", bufs=2, space="PSUM") as psT,
        ):
            # ---- x0 loads at the head of the scalar (ACT) DMA queue ----
            x0 = []
            for dt in range(DT):
                x = x0p.tile([128, BN], BF16, name=f"x0_{dt}")
                nc.scalar.dma_start(
                    x[:], inpT_d[dt * 128:(dt + 1) * 128, :])
                x0.append(x)

            # ---- control-path DMAs on a clear gpsimd queue ----
            eye = constp.tile([128, 128], F32R)
            nc.gpsimd.dma_start(eye[:], eye_d[:].bitcast(F32R))
            oh = constp.tile([JT, N], F32R)
            nc.gpsimd.dma_start(oh[:], oh_d[:].bitcast(F32R))
            adjts = []
            for t in range(JT):
                a = supp.tile([128, N], BF16, name=f"adj{t}", tag="adj",
                              bufs=4)
                nc.gpsimd.dma_start(a[:], adj_d[t * 128:(t + 1) * 128, :])
                adjts.append(a)
            bias_bc = constp.tile([128, D], F32)
            nc.gpsimd.dma_start(
                bias_bc[:], bias_d.unsqueeze(0).broadcast_to([128, D]))

            # ---- weights on sync queue: (W0,W2) pairs gate V0 -> U0 ----
            wtiles = {}
            worder = [(m, dt) for dt in range(DT) for m in (0, 2)] + \
                     [(m, dt) for dt in range(DT) for m in (1, 3)]
            for m, dt in worder:
                w = wp.tile([128, D], F32, name=f"w{m}_{dt}", tag="wt")
                nc.sync.dma_start(
                    w[:], wts_v[m, dt * 128:(dt + 1) * 128, :])
                wtiles[(m, dt)] = w

            # ---- PE warmup: release the HAM clock gate before real work ----
            for k in range(WARMUP_A):
                wps = psA.tile([128, 128], F32, name=f"wa{k}", tag="ps")
                nc.tensor.matmul(wps[:], eye[:], eye[:], start=True,
                                 stop=True)
            for k in range(WARMUP_B):
                wps = psA.tile([128, 512], F32, name=f"wb{k}", tag="ps")
                nc.tensor.matmul(wps[:], x0[0][:, 0:128], x0[0][:, 0:512],
                                 start=True, stop=True)

            # ---------------- support: d column + on-chip broadcast -------
            dcols, dsqs = [], []
            dvec4 = supp.tile([128, JT], F32R, name="dvec4")
            for t in range(JT):
                rs = supp.tile([128, 1], F32, name=f"rs{t}", tag="rs", bufs=2)
                nc.vector.tensor_reduce(rs[:], adjts[t][:], axis=AX.X,
                                        op=ALU.add)
                nc.vector.tensor_scalar_add(rs[:], rs[:], 1.0)
                sq = supp.tile([128, 1], F32, name=f"sq{t}", tag="sq", bufs=2)
                nc.scalar.sqrt(sq[:], rs[:])
                dcol = supp.tile([128, 1], F32, name=f"dcol{t}")
                nc.vector.reciprocal(dcol[:], sq[:])
                nc.vector.tensor_copy(dvec4[:, t:t + 1], dcol[:])
                dsq = supp.tile([128, 1], F32, name=f"dsq{t}")
                nc.vector.tensor_mul(dsq[:], dcol[:], dcol[:])
                dcols.append(dcol)
                dsqs.append(dsq)
            # d as 4 rows: [4, 128] via PE transpose, then broadcast to all
            # 128 partitions with one-hot matmuls.
            ptD = psA.tile([JT, 128], F32R, name="ptD", tag="ps")
            nc.tensor.transpose(ptD[:], dvec4[:], eye[:])
            dT4 = supp.tile([JT, 128], F32R, name="dT4")
            nc.vector.tensor_copy(dT4[:], ptD[:])
            psB = psA.tile([128, 512], F32, name="psB", tag="ps")
            for t in range(JT):
                nc.tensor.matmul(
                    psB[:, t * 128:(t + 1) * 128],
                    oh[:, t * 128:(t + 1) * 128], dT4[:],
                    start=True, stop=True)
            dbc = constp.tile([128, N], F32)
            nc.vector.tensor_copy(dbc[:], psB[:])

            st_t = []
            for t in range(JT):
                s = supp.tile([128, N], F32R, name=f"st{t}")
                nc.vector.scalar_tensor_tensor(
                    s[:], adjts[t][:], dcols[t][:], dbc[:], ALU.mult, ALU.mult)
                dfix = supp.tile([128, 128], F32, name=f"dfix{t}",
                                 tag="dfix", bufs=2)
                nc.vector.tensor_scalar_mul(dfix[:], eye[:].bitcast(F32),
                                            dsqs[t][:])
                nc.vector.tensor_add(
                    s[:, t * 128:(t + 1) * 128],
                    s[:, t * 128:(t + 1) * 128], dfix[:])
                st_t.append(s)

            # ---------------- PE: transpose S^T -> S ----------------
            s_t = [supp.tile([128, N], F32R, name=f"s{t}") for t in range(JT)]
            for src in range(JT):
                for dst in range(JT):
                    pt = psT.tile([128, 128], F32R, name=f"pt{src}_{dst}",
                                  tag="pt")
                    nc.tensor.transpose(
                        pt[:], st_t[src][:, dst * 128:(dst + 1) * 128], eye[:])
                    nc.scalar.copy(s_t[dst][:, src * 128:(src + 1) * 128],
                                   pt[:])

            # ---- V8 m=2 on scalar (early, frees W2); V0 subs on vector ----
            v8 = {}
            for m in (1, 2, 3):
                for dp in range(DP):
                    v8[(m, dp)] = v8p.tile([128, 2, D], F8,
                                           name=f"v8_{m}_{dp}")
            for dt in range(DT):
                nc.scalar.mul(v8[(2, dt // 2)][:, dt % 2, :],
                              wtiles[(2, dt)][:], 2.0 * SV)
            v0 = []
            for dt in range(DT):
                v = v0p.tile([128, D], BF16, name=f"v0_{dt}")
                nc.vector.tensor_sub(v[:], wtiles[(0, dt)][:],
                                     wtiles[(2, dt)][:])
                v0.append(v)

            # ---- V1 tmps (vector) ----
            vtmps = []
            for dt in range(DT):
                tmp = vtp.tile([128, D], BF16, name=f"vt_{dt}", tag="vt",
                               bufs=6)
                nc.vector.scalar_tensor_tensor(
                    tmp[:], wtiles[(3, dt)][:], -3.0, wtiles[(1, dt)][:],
                    ALU.mult, ALU.add)
                vtmps.append(tmp)

            # ---------------- U0 = X @ V0 + bias (bf16) ----------------
            u0 = []
            for rb in range(NT):
                ut = u0p.tile([128, D], BF16, name=f"u0_{rb}")
                u0.append(ut)
                pss = [psA.tile([128, 512], F32, name=f"pu0_{rb}_{e}",
                                tag="ps") for e in range(2)]
                for dt in range(DT):
                    lhs = x0[dt][:, rb * 128:(rb + 1) * 128]
                    for e in range(2):
                        nc.tensor.matmul(
                            pss[e][:, 0:EC], lhs,
                            v0[dt][:, e * EC:(e + 1) * EC],
                            start=(dt == 0), stop=(dt == DT - 1))
                for e in range(2):
                    nc.vector.tensor_add(
                        ut[:, e * EC:(e + 1) * EC], pss[e][:, 0:EC],
                        bias_bc[:, e * EC:(e + 1) * EC])

            # ---- X8 quantize + V8 m=1,3 on scalar (needed by U123) ----
            x8 = []
            for dp in range(DP):
                t8 = x8p.tile([128, 2, BN], F8, name=f"x8_{dp}")
                for i in range(2):
                    nc.scalar.mul(t8[:, i, :], x0[2 * dp + i][:], SX)
                x8.append(t8)
            for dp in range(DP):
                for i in range(2):
                    dt = 2 * dp + i
                    nc.scalar.mul(v8[(1, dp)][:, i, :], vtmps[dt][:], SV)
                    nc.scalar.mul(v8[(3, dp)][:, i, :], wtiles[(3, dt)][:],
                                  4.0 * SV)

            # ---------------- PE: powers (S^T)^2, (S^T)^3 ----------------
            pt8 = {}
            for m in (1, 2, 3):
                for u in range(2):
                    pt8[(m, u)] = pt8p.tile([128, 2, N], F8,
                                            name=f"pt8_{m}_{u}")
            for u in range(2):
                for i in range(2):
                    nc.vector.tensor_scalar_mul(
                        pt8[(1, u)][:, i, :], st_t[2 * u + i][:], SP)
            st2 = [supp.tile([128, N], F32R, name=f"st2_{t}")
                   for t in range(JT)]
            for ab in range(JT):
                ps = psA.tile([128, 512], F32, name=f"p2_{ab}", tag="ps")
                for cb in range(JT):
                    nc.tensor.matmul(
                        ps[:], s_t[cb][:, ab * 128:(ab + 1) * 128],
                        st_t[cb][:], start=(cb == 0), stop=(cb == JT - 1))
                nc.scalar.copy(st2[ab][:], ps[:])
                nc.scalar.mul(pt8[(2, ab // 2)][:, ab % 2, :], ps[:], SP)
            for ab in range(JT):
                ps = psA.tile([128, 512], F32, name=f"p3_{ab}", tag="ps")
                for cb in range(JT):
                    nc.tensor.matmul(
                        ps[:], s_t[cb][:, ab * 128:(ab + 1) * 128],
                        st2[cb][:], start=(cb == 0), stop=(cb == JT - 1))
                nc.scalar.mul(pt8[(3, ab // 2)][:, ab % 2, :], ps[:], SP)

            # ---------------- U_m = X @ V_m (fp8 DoubleRow) ----------------
            u8 = {}
            for m in (1, 2, 3):
                for b in range(BL):
                    for u in range(2):
                        u8[(m, b, u)] = u8p.tile(
                            [128, 2, D], F8, name=f"u8_{m}_{b}_{u}")
            for rb in range(NT):
                b, jt = rb // JT, rb % JT
                u, i = jt // 2, jt % 2
                for m in (1, 2, 3):
                    pss = [psA.tile([128, 512], F32, name=f"pu{m}_{rb}_{e}",
                                    tag="ps") for e in range(2)]
                    for dp in range(DP):
                        lhs = x8[dp][:, :, rb * 128:(rb + 1) * 128]
                        for e in range(2):
                            nc.tensor.matmul(
                                pss[e][:, 0:EC], lhs,
                                v8[(m, dp)][:, :, e * EC:(e + 1) * EC],
                                start=(dp == 0), stop=(dp == DP - 1),
                                perf_mode=DR)
                    dst = u8[(m, b, u)]
                    ev = (nc.vector, nc.scalar)[(rb * 3 + m) % 2]
                    for e in range(2):
                        if ev is nc.scalar:
                            ev.mul(dst[:, i, e * EC:(e + 1) * EC],
                                   pss[e][:, 0:EC], U8SCALE)
                        else:
                            ev.tensor_scalar_mul(
                                dst[:, i, e * EC:(e + 1) * EC],
                                pss[e][:, 0:EC], U8SCALE)

            # ---------------- apply + final eviction ----------------
            MP = [(1, 0), (1, 1), (2, 0), (2, 1), (3, 0), (3, 1)]
            for b in range(BL):
                for nb in range(JT):
                    rb = b * JT + nb
                    pss = [psA.tile([128, 512], F32, name=f"pa_{rb}_{e}",
                                    tag="ps") for e in range(2)]
                    for k, (m, u) in enumerate(MP):
                        lhs = pt8[(m, u)][:, :, nb * 128:(nb + 1) * 128]
                        for e in range(2):
                            nc.tensor.matmul(
                                pss[e][:, 0:EC], lhs,
                                u8[(m, b, u)][:, :, e * EC:(e + 1) * EC],
                                start=(k == 0), stop=(k == len(MP) - 1),
                                perf_mode=DR)
                    so = outp.tile([128, D], F32, name=f"so_{rb}", tag="so")
                    for e in range(2):
                        nc.vector.scalar_tensor_tensor(
                            so[:, e * EC:(e + 1) * EC], pss[e][:, 0:EC],
                            DESCALE, u0[rb][:, e * EC:(e + 1) * EC],
                            ALU.mult, ALU.add)
                    nc.sync.dma_start(
                        out_d[rb * 128:(rb + 1) * 128, :], so[:])
    nc.compile()
    return nc


_CACHE = {}


def _get_program():
    if "nc" not in _CACHE:
        _CACHE["nc"] = _build_program()
    return _CACHE["nc"]


def make_in_maps(inputs, adj, weights, biases):
    inputs = np.ascontiguousarray(inputs, dtype=np.float32)
    adj = np.ascontiguousarray(adj, dtype=np.float32)
    weights = np.ascontiguousarray(weights, dtype=np.float32)
    biases = np.ascontiguousarray(biases, dtype=np.float32)
    assert inputs.shape == (B, N, D)
    assert adj.shape == (N, N)
    assert weights.shape == (D * 4, D)
    assert biases.shape == (D,)
    eye = np.eye(128, dtype=np.float32)
    oh = np.zeros((JT, N), dtype=np.float32)
    for t in range(JT):
        oh[t, t * 128:(t + 1) * 128] = 1.0
    adj8 = adj.astype(ml_dtypes.bfloat16)
    in_maps = []
    for c in range(N_CORES):
        x0T = np.ascontiguousarray(
            inputs[c * BL:(c + 1) * BL].reshape(BN, D).T).astype(
                ml_dtypes.bfloat16)
        in_maps.append({
            "inpT": x0T,
            "adj8": adj8,
            "wts": weights,
            "bias": biases,
            "eye": eye,
            "oh": oh,
        })
    return in_maps


def kernel(inputs, adj, weights, biases):
    nc = _get_program()
    in_maps = make_in_maps(inputs, adj, weights, biases)
    res = run_bass_kernel_spmd(nc, in_maps, list(range(N_CORES)))
    out = np.concatenate(
        [res.results[c]["out"].reshape(BL, N, D) for c in range(N_CORES)],
        axis=0)
    return out


# revision 22
# speedup vs baseline: 1.5477x; 1.2432x over previous
"""DGCN diffusion-graph-conv kernel for 8 Trainium2 NeuronCores (v6).

Math (per the reference):
    support S = D^-1/2 (adj+I)^T D^-1/2,  D = diag(rowsum(adj+I))
    x_m = T_m(S) x0  (Chebyshev, K=3),  out = sum_m x_m @ W_m + bias

Folding the Chebyshev coefficients into the weights
    V0 = W0 - W2, V1 = W1 - 3*W3, V2 = 2*W2, V3 = 4*W3
gives out_b = sum_{m=0..3} S^m (X_b @ V_m).

With M = adj + I and d = rowsum(M)^-1/2, S^m factors as
    S^m = diag(d) (M^T d^2)^(m-1) M^T diag(d)
so defining G_m = M (d^2 M)^(m-1)  (G_1 = M), the apply stage is
    out[n,:] = U0[n,:] + d_n * sum_m sum_j G_m[j,n] * (d_j U_m[j,:]).
The outer diag(d) scales fold into the U eviction (d_j, a per-partition
scalar) and the final eviction (d_n); the powers G_2, G_3 need only
d^2 = 1/rowsum - a vector-engine reciprocal, keeping the whole matrix
pipeline off the scalar-engine sqrt path.

Per-core plan (data-parallel over batch, 4 batches/core):
    1.  M tiles in bf16 from adj; PE-transpose -> M^T; G2 = M d^2 M and
        G3 = M (d^2 M)^2 as bf16 matmuls.
    2.  U0 = X @ V0 + bias in bf16 matmuls (error-critical path).
    3.  U_m = X @ V_m (m=1..3) in fp8e4m3 DoubleRow (2 K-tiles/pass).
        Errors here are suppressed ~20x by the small S^m magnitudes.
    4.  out = U0 + d * (G-apply over stacked d*U) in fp8 DoubleRow.
fp8 scales: X x16, V x32, G_m x32, (d U_m) x128; descale d_n/2^12 is
applied per-partition in the final eviction.

Everything DMA-heavy ships in bf16 (inputs, adj, weights); dummy eye
matmuls warm the PE (HAM clock gate) before the real stream.
"""

import numpy as np
import ml_dtypes

import concourse.bacc as bacc
import concourse.tile as tile
import concourse.mybir as mybir
from concourse.bass_utils import run_bass_kernel_spmd

F32 = mybir.dt.float32
BF16 = mybir.dt.bfloat16
F8 = mybir.dt.float8e4
AX = mybir.AxisListType
ALU = mybir.AluOpType
DR = mybir.MatmulPerfMode.DoubleRow

N_CORES = 8
B, N, D = 32, 512, 768
BL = B // N_CORES          # batches per core = 4
BN = BL * N                # rows per core = 2048
NT = BN // 128             # 16 row blocks
DT = D // 128              # 6 feature tiles
DP = DT // 2               # 3 feature-tile pairs
JT = N // 128              # 4 node tiles
EC = 384                   # output-column chunk (psum-bank safe)

SX, SV, SB, SU = 16.0, 32.0, 32.0, 128.0
DU_EV = SU / (SX * SV)             # 1/4: psum(=512 U) -> d*U*128 with d AP
DESCALE = 1.0 / (SB * SU)          # 2^-12, folded with d_n into final AP

WARMUP_A = 28                      # bf16 eye matmuls to open the HAM gate


def _build_program():
    nc = bacc.Bacc("TRN2", target_bir_lowering=False, debug=False,
                   num_devices=N_CORES)
    inpT_d = nc.dram_tensor("inpT", [D, BN], BF16, kind="ExternalInput").ap()
    adj_d = nc.dram_tensor("adj8", [N, N], BF16, kind="ExternalInput").ap()
    wts_d = nc.dram_tensor("wts16", [D * 4, D], BF16,
                           kind="ExternalInput").ap()
    bias_d = nc.dram_tensor("bias", [D], F32, kind="ExternalInput").ap()
    eye_d = nc.dram_tensor("eye16", [128, 128], BF16,
                           kind="ExternalInput").ap()
    out_d = nc.dram_tensor("out", [BN, D], F32, kind="ExternalOutput").ap()

    wts_v = wts_d.rearrange("(d m) e -> m d e", m=4)

    with tile.TileContext(nc) as tc:
        with (
            tc.tile_pool(name="const", bufs=1) as constp,
            tc.tile_pool(name="x0", bufs=1) as x0p,
            tc.tile_pool(name="x8", bufs=1) as x8p,
            tc.tile_pool(name="wraw", bufs=8) as wp,
            tc.tile_pool(name="v0", bufs=1) as v0p,
            tc.tile_pool(name="v8", bufs=1) as v8p,
            tc.tile_pool(name="vtmp", bufs=6) as vtp,
            tc.tile_pool(name="sup", bufs=1) as supp,
            tc.tile_pool(name="pt8", bufs=1) as pt8p,
            tc.tile_pool(name="u0", bufs=1) as u0p,
            tc.tile_pool(name="u8", bufs=1) as u8p,
            tc.tile_pool(name="outst", bufs=4) as outp,
            tc.tile_pool(name="psA", bufs=6, space="PSUM") as psA,
            tc.tile_pool(name="psT", bufs=2, space="PSUM") as psT,
        ):
            # ---- x0 loads at the head of the scalar (ACT) DMA queue ----
            x0 = []
            for dt in range(DT):
                x = x0p.tile([128, BN], BF16, name=f"x0_{dt}")
                nc.scalar.dma_start(
                    x[:], inpT_d[dt * 128:(dt + 1) * 128, :])
                x0.append(x)

            # ---- control-path DMAs on a clear gpsimd queue ----
            eye = constp.tile([128, 128], BF16)
            nc.gpsimd.dma_start(eye[:], eye_d[:])
            adjts = []
            for t in range(JT):
                a = supp.tile([128, N], BF16, name=f"adj{t}")
                nc.gpsimd.dma_start(a[:], adj_d[t * 128:(t + 1) * 128, :])
                adjts.append(a)
            bias_bc = constp.tile([128, D], F32)
            nc.gpsimd.dma_start(
                bias_bc[:], bias_d.unsqueeze(0).broadcast_to([128, D]))

            # ---- weights on sync queue: (W0,W2) pairs gate V0 -> U0 ----
            wtiles = {}
            worder = [(m, dt) for dt in range(DT) for m in (0, 2)] + \
                     [(m, dt) for dt in range(DT) for m in (1, 3)]
            for m, dt in worder:
                w = wp.tile([128, D], BF16, name=f"w{m}_{dt}", tag="wt")
                nc.sync.dma_start(
                    w[:], wts_v[m, dt * 128:(dt + 1) * 128, :])
                wtiles[(m, dt)] = w

            # ---- PE warmup: release the HAM clock gate before real work ----
            for k in range(WARMUP_A):
                wps = psA.tile([128, 128], F32, name=f"wa{k}", tag="ps")
                nc.tensor.matmul(wps[:], eye[:], eye[:], start=True,
                                 stop=True)

            # ---------------- rowsums, d^2 (vector-only path) -------------
            dsqs, sqs, rss = [], [], []
            for t in range(JT):
                rs = supp.tile([128, 1], F32, name=f"rs{t}")
                nc.vector.tensor_reduce(rs[:], adjts[t][:], axis=AX.X,
                                        op=ALU.add)
                nc.vector.tensor_scalar_add(rs[:], rs[:], 1.0)
                dsq = supp.tile([128, 1], F32, name=f"dsq{t}")
                nc.vector.reciprocal(dsq[:], rs[:])
                sq = supp.tile([128, 1], F32, name=f"sq{t}")
                nc.scalar.sqrt(sq[:], rs[:])
                rss.append(rs)
                dsqs.append(dsq)
                sqs.append(sq)

            # ---------------- M, Mr = d^2 M (bf16) ----------------
            mbs, mrs = [], []
            for t in range(JT):
                mb = supp.tile([128, N], BF16, name=f"mb{t}")
                nc.vector.tensor_copy(mb[:], adjts[t][:])
                nc.vector.tensor_add(
                    mb[:, t * 128:(t + 1) * 128],
                    adjts[t][:, t * 128:(t + 1) * 128], eye[:])
                mbs.append(mb)
            for t in range(JT):
                mr = supp.tile([128, N], BF16, name=f"mr{t}")
                nc.vector.tensor_scalar_mul(mr[:], mbs[t][:], dsqs[t][:])
                mrs.append(mr)

            # ---------------- PE: transpose M -> M^T ----------------
            mts = [supp.tile([128, N], BF16, name=f"mt{t}")
                   for t in range(JT)]
            for src in range(JT):
                for dst in range(JT):
                    pt = psT.tile([128, 128], BF16, name=f"pt{src}_{dst}",
                                  tag="pt")
                    nc.tensor.transpose(
                        pt[:], mbs[src][:, dst * 128:(dst + 1) * 128], eye[:])
                    nc.scalar.copy(mts[dst][:, src * 128:(src + 1) * 128],
                                   pt[:])

            # ---------------- PE: G2 = M d2 M, G3 = M (d2 M)^2 ------------
            pt8 = {}
            for m in (1, 2, 3):
                for u in range(2):
                    pt8[(m, u)] = pt8p.tile([128, 2, N], F8,
                                            name=f"pt8_{m}_{u}")
            for t in range(JT):
                nc.vector.tensor_scalar_mul(
                    pt8[(1, t // 2)][:, t % 2, :], mbs[t][:], SB)
            g2r = [supp.tile([128, N], BF16, name=f"g2r{t}")
                   for t in range(JT)]
            for ab in range(JT):
                ps = psA.tile([128, 512], F32, name=f"p2_{ab}", tag="ps")
                for cb in range(JT):
                    nc.tensor.matmul(
                        ps[:], mts[cb][:, ab * 128:(ab + 1) * 128],
                        mrs[cb][:], start=(cb == 0), stop=(cb == JT - 1))
                nc.scalar.mul(g2r[ab][:], ps[:], dsqs[ab][:])
                nc.scalar.mul(pt8[(2, ab // 2)][:, ab % 2, :], ps[:], SB)
            for ab in range(JT):
                ps = psA.tile([128, 512], F32, name=f"p3_{ab}", tag="ps")
                for cb in range(JT):
                    nc.tensor.matmul(
                        ps[:], mts[cb][:, ab * 128:(ab + 1) * 128],
                        g2r[cb][:], start=(cb == 0), stop=(cb == JT - 1))
                nc.scalar.mul(pt8[(3, ab // 2)][:, ab % 2, :], ps[:], SB)

            # ---- d column + eviction-scale columns (vector) ----
            du_ev, dn_ev = [], []
            for t in range(JT):
                dcol = supp.tile([128, 1], F32, name=f"dcol{t}")
                nc.vector.reciprocal(dcol[:], sqs[t][:])
                du = supp.tile([128, 1], F32, name=f"du{t}")
                nc.vector.tensor_scalar_mul(du[:], dcol[:], DU_EV)
                dn = supp.tile([128, 1], F32, name=f"dn{t}")
                nc.vector.tensor_scalar_mul(dn[:], dcol[:], DESCALE)
                du_ev.append(du)
                dn_ev.append(dn)

            # ---- V8 m=2 on scalar (early, frees W2); V0 subs on vector ----
            v8 = {}
            for m in (1, 2, 3):
                for dp in range(DP):
                    v8[(m, dp)] = v8p.tile([128, 2, D], F8,
                                           name=f"v8_{m}_{dp}")
            for dt in range(DT):
                nc.scalar.mul(v8[(2, dt // 2)][:, dt % 2, :],
                              wtiles[(2, dt)][:], 2.0 * SV)
            v0 = []
            for dt in range(DT):
                v = v0p.tile([128, D], BF16, name=f"v0_{dt}")
                nc.vector.tensor_sub(v[:], wtiles[(0, dt)][:],
                                     wtiles[(2, dt)][:])
                v0.append(v)
            vtmps = []
            for dt in range(DT):
                tmp = vtp.tile([128, D], BF16, name=f"vt_{dt}", tag="vt")
                nc.vector.scalar_tensor_tensor(
                    tmp[:], wtiles[(3, dt)][:], -3.0, wtiles[(1, dt)][:],
                    ALU.mult, ALU.add)
                vtmps.append(tmp)

            # ---------------- U0 = X @ V0 + bias (bf16) ----------------
            u0 = []
            for rb in range(NT):
                ut = u0p.tile([128, D], BF16, name=f"u0_{rb}")
                u0.append(ut)
                pss = [psA.tile([128, 512], F32, name=f"pu0_{rb}_{e}",
                                tag="ps") for e in range(2)]
                for dt in range(DT):
                    lhs = x0[dt][:, rb * 128:(rb + 1) * 128]
                    for e in range(2):
                        nc.tensor.matmul(
                            pss[e][:, 0:EC], lhs,
                            v0[dt][:, e * EC:(e + 1) * EC],
                            start=(dt == 0), stop=(dt == DT - 1))
                for e in range(2):
                    nc.vector.tensor_add(
                        ut[:, e * EC:(e + 1) * EC], pss[e][:, 0:EC],
                        bias_bc[:, e * EC:(e + 1) * EC])

            # ---- X8 quantize + V8 m=1,3 on scalar (needed by U123) ----
            x8 = []
            for dp in range(DP):
                t8 = x8p.tile([128, 2, BN], F8, name=f"x8_{dp}")
                for i in range(2):
                    nc.scalar.mul(t8[:, i, :], x0[2 * dp + i][:], SX)
                x8.append(t8)
            for dp in range(DP):
                for i in range(2):
                    dt = 2 * dp + i
                    nc.scalar.mul(v8[(1, dp)][:, i, :], vtmps[dt][:], SV)
                    nc.scalar.mul(v8[(3, dp)][:, i, :], wtiles[(3, dt)][:],
                                  4.0 * SV)

            # ---------------- U_m = X @ V_m (fp8 DoubleRow) ----------------
            u8 = {}
            for m in (1, 2, 3):
                for b in range(BL):
                    for u in range(2):
                        u8[(m, b, u)] = u8p.tile(
                            [128, 2, D], F8, name=f"u8_{m}_{b}_{u}")
            for rb in range(NT):
                b, jt = rb // JT, rb % JT
                u, i = jt // 2, jt % 2
                for m in (1, 2, 3):
                    pss = [psA.tile([128, 512], F32, name=f"pu{m}_{rb}_{e}",
                                    tag="ps") for e in range(2)]
                    for dp in range(DP):
                        lhs = x8[dp][:, :, rb * 128:(rb + 1) * 128]
                        for e in range(2):
                            nc.tensor.matmul(
                                pss[e][:, 0:EC], lhs,
                                v8[(m, dp)][:, :, e * EC:(e + 1) * EC],
                                start=(dp == 0), stop=(dp == DP - 1),
                                perf_mode=DR)
                    dst = u8[(m, b, u)]
                    ev = (nc.vector, nc.scalar)[(rb * 3 + m) % 2]
                    for e in range(2):
                        if ev is nc.scalar:
                            ev.mul(dst[:, i, e * EC:(e + 1) * EC],
                                   pss[e][:, 0:EC], du_ev[jt][:])
                        else:
                            ev.tensor_scalar_mul(
                                dst[:, i, e * EC:(e + 1) * EC],
                                pss[e][:, 0:EC], du_ev[jt][:])

            # ---------------- apply + final eviction ----------------
            MP = [(1, 0), (1, 1), (2, 0), (2, 1), (3, 0), (3, 1)]
            for b in range(BL):
                for nb in range(JT):
                    rb = b * JT + nb
                    pss = [psA.tile([128, 512], F32, name=f"pa_{rb}_{e}",
                                    tag="ps") for e in range(2)]
                    for k, (m, u) in enumerate(MP):
                        lhs = pt8[(m, u)][:, :, nb * 128:(nb + 1) * 128]
                        for e in range(2):
                            nc.tensor.matmul(
                                pss[e][:, 0:EC], lhs,
                                u8[(m, b, u)][:, :, e * EC:(e + 1) * EC],
                                start=(k == 0), stop=(k == len(MP) - 1),
                                perf_mode=DR)
                    so = outp.tile([128, D], F32, name=f"so_{rb}", tag="so")
                    for e in range(2):
                        nc.vector.scalar_tensor_tensor(
                            so[:, e * EC:(e + 1) * EC], pss[e][:, 0:EC],
                            dn_ev[nb][:], u0[rb][:, e * EC:(e + 1) * EC],
                            ALU.mult, ALU.add)
                    nc.sync.dma_start(
                        out_d[rb * 128:(rb + 1) * 128, :], so[:])
    nc.compile()
    return nc


_CACHE = {}


def _get_program():
    if "nc" not in _CACHE:
        _CACHE["nc"] = _build_program()
    return _CACHE["nc"]


def make_in_maps(inputs, adj, weights, biases):
    inputs = np.ascontiguousarray(inputs, dtype=np.float32)
    adj = np.ascontiguousarray(adj, dtype=np.float32)
    weights = np.ascontiguousarray(weights, dtype=np.float32)
    biases = np.ascontiguousarray(biases, dtype=np.float32)
    assert inputs.shape == (B, N, D)
    assert adj.shape == (N, N)
    assert weights.shape == (D * 4, D)
    assert biases.shape == (D,)
    eye = np.eye(128, dtype=ml_dtypes.bfloat16)
    adj8 = adj.astype(ml_dtypes.bfloat16)
    wts16 = weights.astype(ml_dtypes.bfloat16)
    in_maps = []
    for c in range(N_CORES):
        x0T = np.ascontiguousarray(
            inputs[c * BL:(c + 1) * BL].reshape(BN, D).T).astype(
                ml_dtypes.bfloat16)
        in_maps.append({
            "inpT": x0T,
            "adj8": adj8,
            "wts16": wts16,
            "bias": biases,
            "eye16": eye,
        })
    return in_maps


def kernel(inputs, adj, weights, biases):
    nc = _get_program()
    in_maps = make_in_maps(inputs, adj, weights, biases)
    res = run_bass_kernel_spmd(nc, in_maps, list(range(N_CORES)))
    out = np.concatenate(
        [res.results[c]["out"].reshape(BL, N, D) for c in range(N_CORES)],
        axis=0)
    return out
